# revision 57
# baseline (speedup 1.0000x reference)
"""GroupedQueryAttention+RoPE Trainium2 kernel (8 NeuronCores), bf16 edition.

Model: d_model=2048, H=32 q-heads, Hkv=8 kv-heads (G=4), head_dim=64,
B=2, T=2048, causal, softmax without max-subtraction (scores are O(6)).

Sharding: 2-way data parallel on batch x 4-way tensor parallel on heads.
Core c: batch = c//4, kv heads {2j, 2j+1} (j=c%4), q heads [8j:8j+8].
Per-core out-proj is row-sharded; host sums the 4 partials per batch.

Key design points (vs the f32r baseline):
  * All matmul operands in bf16 (PSUM accumulation stays f32): halves DMA
    bytes, lifts the 4-cycles/row penalty on <256-free f32r matmuls (v-proj).
  * Every DRAM input is host-packed to the exact SBUF layout so each logical
    load is ONE contiguous DMA (HWDGE is a serial ~630ns/DMA resource):
    4 x-chunk loads + 3 weight loads + 1 consts load + 4 out stores +
    4 stagB moves = 16 DMAs total (baseline: 343).
  * RoPE half-swap done by a PE permutation matmul (no DMAs).
  * exp over both heads of a pair in one ACT op ([128, 2, 512] PSUM tile
    spanning 2 banks); causal-diagonal tiles computed at restricted width.
  * out-proj partials written as bf16; host sums in f32.
"""
import numpy as np

D = 2048
T = 2048
B = 2
HD = 64
HALF = 32
THETA = 10000.0
NC = 8
TC = 4          # t-chunks of 512
CH = 512        # chunk width
KT = 16         # 128-row k-tiles per projection contraction

_compiled = None
_trace = False
_trace_sim = False
_last = None


def _build():
    import concourse.bacc as bacc
    import concourse.mybir as mybir
    from concourse.tile import TileContext

    F32 = mybir.dt.float32
    BF16 = mybir.dt.bfloat16
    Exp = mybir.ActivationFunctionType.Exp

    nc = bacc.Bacc("TRN2", target_bir_lowering=False, debug=False, num_devices=NC)

    xs_d = nc.dram_tensor("xs", [128, TC, KT, CH], BF16, kind="ExternalInput")
    wq_d = nc.dram_tensor("wq", [128, KT, 512], BF16, kind="ExternalInput")
    wkv_d = nc.dram_tensor("wkv", [128, KT, 256], BF16, kind="ExternalInput")
    wo_d = nc.dram_tensor("wo", [128, 4, D], BF16, kind="ExternalInput")
    # consts: [crep 2048 | srep 2048 | masks 2048 | perm 128]
    consts_d = nc.dram_tensor("consts", [128, 6272], BF16, kind="ExternalInput")
    out_d = nc.dram_tensor("out", [128, TC, 4, D], BF16, kind="ExternalOutput")

    with TileContext(nc, trace_sim=_trace_sim) as tc:
        with (
            tc.tile_pool(name="consts", bufs=1) as consts,
            tc.tile_pool(name="wpool", bufs=1) as wpool,
            tc.tile_pool(name="xpool", bufs=2) as xpool,
            tc.tile_pool(name="qtpool", bufs=4) as qtpool,
            tc.tile_pool(name="ktpool", bufs=4) as ktpool,
            tc.tile_pool(name="vpool", bufs=4) as vpool,
            tc.tile_pool(name="rpool", bufs=4) as rpool,
            tc.tile_pool(name="ptpool", bufs=4) as ptpool,
            tc.tile_pool(name="nrm", bufs=2) as nrm,
            tc.tile_pool(name="cqpool", bufs=2) as cqpool,
            tc.tile_pool(name="sbpool", bufs=2) as sbpool,
            tc.tile_pool(name="opool", bufs=2) as opool,
            tc.tile_pool(name="mmps", bufs=2, space="PSUM") as mmps,
            tc.tile_pool(name="spps", bufs=2, space="PSUM") as spps,
            tc.tile_pool(name="ctxps", bufs=1, space="PSUM") as ctxps,
        ):
            # ---- resident weights / constants, ordered for fastest
            # time-to-first-matmul: k/v weights -> tables -> x0 -> q weights
            # preamble DMAs, ordered so PE's first matmul starts early and
            # the q-weights land just in time after k/v
            wkv_t = wpool.tile([128, KT, 256], BF16)
            nc.sync.dma_start(out=wkv_t[:, :, 0:128], in_=wkv_d[:, :, 0:128])
            xts = {}
            xts[0] = xpool.tile([128, KT, CH], BF16, tag="xt", name="xt_0")
            for q4 in range(4):
                nc.sync.dma_start(out=xts[0][:, 4 * q4:4 * q4 + 4, :],
                                  in_=xs_d[:, 0, 4 * q4:4 * q4 + 4, :])
                if q4 == 1:
                    nc.sync.dma_start(out=wkv_t[:, :, 128:256],
                                      in_=wkv_d[:, :, 128:256])
            wq_t = wpool.tile([128, KT, 512], BF16)
            nc.sync.dma_start(out=wq_t[:, :, 0:256], in_=wq_d[:, :, 0:256])
            ct = consts.tile([128, 6272], BF16)
            nc.sync.dma_start(out=ct[:, 0:4096], in_=consts_d[:, 0:4096])
            nc.sync.dma_start(out=ct[:, 6144:6272], in_=consts_d[:, 6144:6272])
            nc.sync.dma_start(out=wq_t[:, :, 256:512], in_=wq_d[:, :, 256:512])
            xts[1] = xpool.tile([128, KT, CH], BF16, tag="xt", name="xt_1")
            nc.sync.dma_start(out=xts[1], in_=xs_d[:, 1, :, :])
            nc.sync.dma_start(out=ct[:, 4096:6144], in_=consts_d[:, 4096:6144])
            wo_t = wpool.tile([128, 4, D], BF16)
            nc.sync.dma_start(out=wo_t, in_=wo_d[:, :, :])
            crep = ct[:, 0:2048]
            srep = ct[:, 2048:4096]
            # mask block r lives at columns [4096 + r*CH, 4096 + (r+1)*CH)
            perm = ct[:, 6144:6272]

            kts = []   # kT tiles per tchunk ([e, t] layout)
            vts = []   # v_aug tiles per tchunk ([t, e|1] layout)
            qts = []   # q tiles per tchunk

            def rope(dst, ps, tcix, tag):
                """dst (bf16 SBUF) = rope(ps [128,CH] f32 PSUM); clobbers ps."""
                sl = slice(tcix * CH, (tcix + 1) * CH)
                qs = rpool.tile([128, CH], BF16, tag="qs", name=f"qs_{tag}")
                nc.vector.tensor_copy(qs, ps[:, :])
                # half-swap via PE permutation (overwrites ps in place)
                nc.tensor.matmul(ps[:, :], perm, qs, start=True, stop=True)
                tcos = rpool.tile([128, CH], BF16, tag="tc", name=f"tc_{tag}")
                nc.vector.tensor_mul(tcos, qs, crep[:, sl])
                tsin = rpool.tile([128, CH], BF16, tag="ts", name=f"ts_{tag}")
                nc.vector.tensor_mul(tsin, ps[:, :], srep[:, sl])
                nc.vector.tensor_add(dst, tcos, tsin)

            # ================= phase A: projections ==========================
            def proj_chunk(tcix):
                if tcix in xts:
                    xt = xts[tcix]
                else:
                    xt = xpool.tile([128, KT, CH], BF16, tag="xt",
                                    name=f"xt_{tcix}")
                    nc.sync.dma_start(out=xt, in_=xs_d[:, tcix, :, :])

                # k projection -> [e(128), t(512)].  Chunk 0's k/v psums come
                # from the score ring (idle until attention starts) so the
                # q projections never wait on rope/v drains at startup.
                if tcix == 0:
                    psK = spps.tile([128, 2, CH], F32, tag="sp",
                                    name=f"psK_{tcix}")[:, 0, :]
                else:
                    psK = mmps.tile([128, CH], F32, tag="mm",
                                    name=f"psK_{tcix}")[:, :]
                for k in range(KT):
                    nc.tensor.matmul(psK, wkv_t[:, k, 0:128], xt[:, k, :],
                                     start=(k == 0), stop=(k == KT - 1))
                kTt = ktpool.tile([128, CH], BF16, tag="kTt", name=f"kTt_{tcix}")
                kts.append(kTt)
                rope(kTt[:, :], psK, tcix, f"k_{tcix}")

                # v projection -> [t(128 x4), e(128)] then augment with ones
                if tcix == 0:
                    psV = spps.tile([128, 2, CH], F32, tag="sp",
                                    name=f"psV_{tcix}")[:, 0, :]
                else:
                    psV = mmps.tile([128, CH], F32, tag="mm",
                                    name=f"psV_{tcix}")[:, :]
                for k in range(KT):
                    for vt in range(4):
                        nc.tensor.matmul(
                            psV[:, vt * 128:(vt + 1) * 128],
                            xt[:, k, vt * 128:(vt + 1) * 128],
                            wkv_t[:, k, 128:256],
                            start=(k == 0 and vt == 0), stop=(k == KT - 1))
                v_augt = vpool.tile([128, 4, 130], BF16, tag="vat",
                                    name=f"vat_{tcix}")
                vts.append(v_augt)

                def v_drain():
                    for vt in range(4):
                        op = (nc.vector.tensor_copy if vt % 2 == 0
                              else nc.scalar.copy)
                        op(v_augt[:, vt, 0:64], psV[:, vt * 128:vt * 128 + 64])
                        op(v_augt[:, vt, 65:129],
                           psV[:, vt * 128 + 64:vt * 128 + 128])
                    nc.vector.memset(v_augt[:, :, 64:65], 1.0)
                    nc.vector.memset(v_augt[:, :, 129:130], 1.0)

                if tcix != 0:
                    v_drain()

                # q projections: pair p -> [e(128 = headA|headB), t(512)]
                qTt = qtpool.tile([128, 4, CH], BF16, tag="qTt", name=f"qTt_{tcix}")
                qts.append(qTt)
                for p in range(4):
                    psQ = mmps.tile([128, CH], F32, tag="mm",
                                    name=f"psQ_{tcix}_{p}")
                    for k in range(KT):
                        nc.tensor.matmul(psQ[:, :],
                                         wq_t[:, k, p * 128:(p + 1) * 128],
                                         xt[:, k, :],
                                         start=(k == 0), stop=(k == KT - 1))
                    rope(qTt[:, p, :], psQ, tcix, f"q{p}_{tcix}")
                    if tcix == 0 and p == 0:
                        v_drain()

            # chunks 2/3's projections are deferred into phase B: attention
            # chunks 0/1 don't need them, and they fill attention bubbles
            for tcix in range(2):
                proj_chunk(tcix)

            # ================= phase B: attention + out-proj ================
            def outproj_block(tcix, ctxq, og, i, fine_store=False):
                """Emit t-block i of chunk tcix's out-projection."""
                for dm in range(4):
                    ops = mmps.tile([128, CH], F32, tag="mm",
                                    name=f"ops_{tcix}_{i}_{dm}")
                    for g in range(4):
                        nc.tensor.matmul(ops[:, :],
                                         ctxq[:, g, i * 128:(i + 1) * 128],
                                         wo_t[:, g, dm * CH:(dm + 1) * CH],
                                         start=(g == 0), stop=(g == 3))
                    if dm % 2 == 0:
                        nc.vector.tensor_copy(
                            og[:, i, dm * CH:(dm + 1) * CH], ops[:, :])
                    else:
                        nc.scalar.copy(
                            og[:, i, dm * CH:(dm + 1) * CH], ops[:, :])
                    if fine_store:
                        nc.sync.dma_start(
                            out=out_d[:, tcix, i, dm * CH:(dm + 1) * CH],
                            in_=og[:, i, dm * CH:(dm + 1) * CH])
                if not fine_store:
                    nc.sync.dma_start(out=out_d[:, tcix, i, :], in_=og[:, i, :])

            # Chunks processed in REVERSE: attention(3) has the most kt work
            # and its kt loop consumes kv chunks in projection order, so it
            # overlaps phase A; later (smaller) chunks get out-proj filler.
            prev = None   # (tcix, ctxq, og) of the chunk whose out-proj is due
            order = list(range(TC))
            for ci, tcix in enumerate(order):
                last_chunk = (ci == TC - 1)
                nkt = 4 * tcix + 4
                qTt = qts[tcix]
                ctxq = cqpool.tile([128, 4, CH], BF16, tag="cq",
                                   name=f"cq_{tcix}")
                og = opool.tile([128, 4, D], BF16, tag="og", name=f"og_{tcix}")
                sbB = sbpool.tile([64, 4, CH], BF16, tag="sb", name=f"sb_{tcix}")
                for p in range(4):
                    ctx = ctxps.tile([65, 2, CH], F32, tag="ctx",
                                     name=f"ctx_{tcix}_{p}")
                    for kt in range(nkt):
                        ktile = kts[kt // 4]
                        vtile = vts[kt // 4]
                        ksl = slice((kt % 4) * 128, (kt % 4 + 1) * 128)
                        r = kt - 4 * tcix
                        lo = 128 * r if r >= 1 else 0
                        sl = slice(lo, CH)
                        sp = spps.tile([128, 2, CH], F32, tag="sp",
                                       name=f"sp_{tcix}_{p}_{kt}")
                        nc.tensor.matmul(sp[:, 0, sl], ktile[0:64, ksl],
                                         qTt[0:64, p, sl],
                                         start=True, stop=True,
                                         tile_position=(0, 0))
                        nc.tensor.matmul(sp[:, 1, sl], ktile[64:128, ksl],
                                         qTt[64:128, p, sl],
                                         start=True, stop=True,
                                         tile_position=(64, 0))
                        pt = ptpool.tile([128, 2, CH], BF16, tag="pt",
                                         name=f"pt_{tcix}_{p}_{kt}")
                        nc.scalar.activation(pt[:, :, sl], sp[:, :, sl], Exp,
                                             scale=0.125)
                        if r >= 0:
                            msl = slice(4096 + r * CH + lo, 4096 + (r + 1) * CH)
                            mab = ct[:, msl].unsqueeze(1).broadcast_to(
                                (128, 2, CH - lo))
                            nc.vector.tensor_mul(pt[:, :, sl], pt[:, :, sl],
                                                 mab)
                        st, spf = (kt == 0), (kt == nkt - 1)
                        nc.tensor.matmul(ctx[:, 0, sl], vtile[:, kt % 4, 0:65],
                                         pt[:, 0, sl], start=st, stop=spf,
                                         skip_group_check=True)
                        nc.tensor.matmul(ctx[:, 1, sl], vtile[:, kt % 4, 65:130],
                                         pt[:, 1, sl], start=st, stop=spf,
                                         skip_group_check=True)
                    final = last_chunk and p == 3
                    if final:
                        # chunk-2's final out-proj block, then the tail: fill
                        # PE with the last chunk's out-proj partial sums
                        # (pairs 0-2) while p3 finishes. Tail psums alternate
                        # between the mm ring and the (now idle) score ring
                        # for a 4-deep evacuation pipeline.
                        def tail_psum(idx, nm):
                            if idx % 2 == 0:
                                return mmps.tile([128, CH], F32, tag="mm",
                                                 name=nm)
                            t2 = spps.tile([128, 2, CH], F32, tag="sp",
                                           name=nm)
                            return t2[:, 0, :]

                        if prev is not None:
                            outproj_block(prev[0], prev[1], prev[2], 3)
                        for i in range(4):
                            for dm in range(4):
                                ops = tail_psum(i * 4 + dm, f"op1_{i}_{dm}")
                                for g in range(3):
                                    nc.tensor.matmul(
                                        ops,
                                        ctxq[:, g, i * 128:(i + 1) * 128],
                                        wo_t[:, g, dm * CH:(dm + 1) * CH],
                                        start=(g == 0), stop=(g == 2))
                                if dm % 2 == 0:
                                    nc.vector.tensor_copy(
                                        og[:, i, dm * CH:(dm + 1) * CH], ops)
                                else:
                                    nc.scalar.copy(
                                        og[:, i, dm * CH:(dm + 1) * CH], ops)
                            nc.sync.dma_start(out=out_d[:, tcix, i, :],
                                              in_=og[:, i, :])
                        # normalize p3 straight from PSUM (no p+1 wants it)
                        rec = nrm.tile([1, 2, CH], F32, tag="rec", name="rc_f")
                        nc.vector.reciprocal(rec, ctx[64:65, :, :])
                        bc = nrm.tile([64, 2, CH], F32, tag="bc", name="bc_f")
                        nc.gpsimd.partition_broadcast(bc, rec[0:1, :, :])
                        nc.vector.tensor_mul(ctxq[0:64, p, :], ctx[0:64, 0, :],
                                             bc[:, 0, :])
                        nc.vector.tensor_mul(sbB[:, p, :], ctx[0:64, 1, :],
                                             bc[:, 1, :])
                        nc.sync.dma_start(out=ctxq[64:128, p, :],
                                          in_=sbB[:, p, :])
                        continue
                    # evacuate PSUM fast (frees the ctx bank for p+1), then
                    # normalize from SBUF: rows 0-63 ctx, row 64 denominator
                    cx = nrm.tile([65, 2, CH], F32, tag="cx",
                                  name=f"cx_{tcix}_{p}")
                    nc.vector.tensor_copy(cx, ctx[:, :, :])
                    rec = nrm.tile([1, 2, CH], F32, tag="rec",
                                   name=f"rc_{tcix}_{p}")
                    nc.vector.reciprocal(rec, cx[64:65, :, :])
                    bc = nrm.tile([64, 2, CH], F32, tag="bc",
                                  name=f"bc_{tcix}_{p}")
                    nc.gpsimd.partition_broadcast(bc, rec[0:1, :, :])
                    nc.vector.tensor_mul(ctxq[0:64, p, :], cx[0:64, 0, :],
                                         bc[:, 0, :])
                    nc.vector.tensor_mul(sbB[:, p, :], cx[0:64, 1, :],
                                         bc[:, 1, :])
                    # move B-head into partitions 64-127 per pair, so the
                    # out-proj g-accumulation can start before later pairs
                    nc.sync.dma_start(out=ctxq[64:128, p, :], in_=sbB[:, p, :])
                    # previous chunk's out-proj, one t-block per pair: emitted
                    # here (later priority) so it fills p-boundary PE stalls
                    if prev is not None:
                        outproj_block(prev[0], prev[1], prev[2], p)

                prev = (tcix, ctxq, og)
                if ci == 0:
                    proj_chunk(2)
                elif ci == 1:
                    proj_chunk(3)

            # tail piece 2: pair-3 contribution, combined by accumulating DMA
            tcix, ctxq, og = prev
            for i in range(4):
                for dm in range(4):
                    if (i * 4 + dm) % 2 == 0:
                        ops = mmps.tile([128, CH], F32, tag="mm",
                                        name=f"op2_{i}_{dm}")
                    else:
                        ops = spps.tile([128, 2, CH], F32, tag="sp",
                                        name=f"op2_{i}_{dm}")[:, 0, :]
                    nc.tensor.matmul(ops,
                                     ctxq[:, 3, i * 128:(i + 1) * 128],
                                     wo_t[:, 3, dm * CH:(dm + 1) * CH],
                                     start=True, stop=True)
                    if dm % 2 == 0:
                        nc.vector.tensor_copy(
                            og[:, i, dm * CH:(dm + 1) * CH], ops)
                    else:
                        nc.scalar.copy(og[:, i, dm * CH:(dm + 1) * CH], ops)
                nc.gpsimd.dma_start(out=out_d[:, tcix, i, :], in_=og[:, i, :],
                                    accum_op=mybir.AluOpType.add)

    nc.compile()
    return nc


def _host_tables():
    inv_freq = 1.0 / (THETA ** (np.arange(0, HD, 2, dtype=np.float64) / HD))
    t = np.arange(T, dtype=np.float64)
    freqs = np.outer(t, inv_freq)          # (T, 32)
    cos = np.cos(freqs)
    sin = np.sin(freqs)
    crep = np.empty((128, T), np.float32)
    srep = np.empty((128, T), np.float32)
    for blk in range(4):                   # 4 blocks of 32 partitions
        j = np.arange(HALF)
        crep[blk * 32:(blk + 1) * 32] = cos[:, j].T
        sgn = -1.0 if (blk % 2 == 0) else 1.0
        srep[blk * 32:(blk + 1) * 32] = sgn * sin[:, j].T
    masks = np.zeros((128, 4 * CH), np.float32)
    tk = np.arange(128)[:, None]
    tq = np.arange(CH)[None, :]
    for r in range(4):
        masks[:, r * CH:(r + 1) * CH] = (tk + 128 * r <= tq).astype(np.float32)
    # rope half-swap permutation: perm[p, i] = 1 iff p == sigma(i),
    # sigma swaps 32-blocks within each 64-block
    perm = np.zeros((128, 128), np.float32)
    i = np.arange(128)
    sigma = (i // 64) * 64 + ((i % 64) + 32) % 64
    perm[sigma, i] = 1.0
    return crep, srep, masks, perm


def kernel(x, Wq, Wk, Wv, Wo, b_out):
    global _compiled, _last
    import ml_dtypes
    from concourse.bass_utils import run_bass_kernel_spmd

    BF = ml_dtypes.bfloat16
    x = np.asarray(x, np.float32)
    Wq = np.asarray(Wq, np.float32)
    Wk = np.asarray(Wk, np.float32)
    Wv = np.asarray(Wv, np.float32)
    Wo = np.asarray(Wo, np.float32)
    b_out = np.asarray(b_out, np.float32)

    crep, srep, masks, perm = _host_tables()
    consts = np.concatenate([crep, srep, masks, perm], axis=1).astype(BF)

    in_maps = []
    for c in range(NC):
        b, j = c // 4, c % 4
        # q-head pair layout: pair p = (head 8j+p, head 8j+p+4)
        qcols = []
        for p in range(4):
            qcols.append(Wq[:, 64 * (8 * j + p):64 * (8 * j + p) + 64])
            qcols.append(Wq[:, 64 * (8 * j + p + 4):64 * (8 * j + p + 4) + 64])
        wq_c = np.concatenate(qcols, axis=1)                       # [2048, 512]
        wk_c = Wk[:, 128 * j:128 * (j + 1)]                        # [2048, 128]
        wv_c = Wv[:, 128 * j:128 * (j + 1)]                        # [2048, 128]
        wkv_c = np.concatenate([wk_c, wv_c], axis=1)               # [2048, 256]
        worows = []
        for g in range(4):
            worows.append(Wo[64 * (8 * j + g):64 * (8 * j + g) + 64, :])
            worows.append(Wo[64 * (8 * j + g + 4):64 * (8 * j + g + 4) + 64, :])
        wo_c = np.concatenate(worows, axis=0)                      # [512, 2048]

        xT = np.ascontiguousarray(x[b].T)                          # [2048, 2048]
        xs = xT.reshape(KT, 128, TC, CH).transpose(1, 2, 0, 3)     # [128,4,16,512]
        in_maps.append({
            "xs": np.ascontiguousarray(xs).astype(BF),
            "wq": np.ascontiguousarray(
                wq_c.reshape(KT, 128, 512).transpose(1, 0, 2)).astype(BF),
            "wkv": np.ascontiguousarray(
                wkv_c.reshape(KT, 128, 256).transpose(1, 0, 2)).astype(BF),
            "wo": np.ascontiguousarray(
                wo_c.reshape(4, 128, D).transpose(1, 0, 2)).astype(BF),
            "consts": consts,
        })

    if _compiled is None:
        _compiled = _build()

    res = run_bass_kernel_spmd(_compiled, in_maps, core_ids=list(range(NC)),
                               trace=_trace)
    _last = res

    full = np.empty((B, T, D), np.float32)
    for b in range(B):
        acc = None
        for j in range(4):
            o = np.asarray(res.results[4 * b + j]["out"]).astype(np.float32)
            o = o.transpose(1, 2, 0, 3).reshape(T, D)   # [p,tc,tt,col]->[t,col]
            acc = o if acc is None else acc + o
        full[b] = acc + b_out[None, :]
    return full
